# revision 1
# baseline (speedup 1.0000x reference)
"""Ewald reciprocal-space sum on 8 Trainium2 NeuronCores.

Math: for each system b, S(k) = sum_n q_n e^{i k.r_n} over the static
integer k-grid n in [-10,10]^3, k = n @ G, G = 2*pi*inv(cell)^T.
Key identity: k.r = n1*phi1 + n2*phi2 + n3*phi3 with phi_d = G_d . r,
so e^{i k.r} factorizes into per-dimension phase tables. Only the
n1 >= 0 half-grid is needed (hemisphere mask kills n1 < 0).

Device work per core (SPMD, core c owns half the atoms of system c//2):
  - phases phi'_d = frac((r @ inv(cell))_d) come in pre-reduced (turns)
  - theta'[j,d] = j * phi'_d  for j in [-10..10]        (DVE)
  - reduce mod 1 into [0,1) turn space                  (DVE/GPSIMD)
  - sin/cos via ACT Sin(2*pi*t - pi)                    (ACT)
  - pair table A = e^{i(n2*phi2+n3*phi3)}  [atoms,441]  (ACT)
  - S partial = (q*e^{i n1 phi1})^T @ A via 2 PSUM-accumulated
    matmuls per 128-atom chunk                          (PE)
Host: tiny O(B*K) weight mask + final reduction (exactly mirrors the
reference), summing partial S across the core pair before squaring.
"""

import numpy as np

# ---- problem constants (hardcoded per contract) ----
B = 4
N_PER = 2000
NK = 10                      # k-grid extent: n in [-NK, NK]
NJ = 2 * NK + 1              # 21
NPAIR = NJ * NJ              # 441
NH = NK + 1                  # 11 non-negative n1 values
DL = 2.0
SIGMA = 1.0
EPS = 1e-6
NORM = 90.0474
TWOPI = 2.0 * np.pi

MAGIC = 12582912.0           # 1.5 * 2**23: fp32 round-to-nearest trick
NPP = NPAIR + 1              # 442: fp32r matmul needs even free counts

N_CORES = 8
CORES_PER_SYS = 2
ATOMS_PER_CORE = (B * N_PER) // N_CORES     # 1000
CHUNKS = 8                                  # ceil(1000/128)
PADN = CHUNKS * 128                         # 1024

_CACHE = {}


def _build_nc():
    import concourse.bacc as bacc
    import concourse.mybir as mybir
    import concourse.tile as tile

    # cheaper TileContext exit: the Bass preamble re-clears the whole
    # kernel sem range at every execution, so the exit-time sem clear and
    # second all-engine barrier are redundant for this single-context
    # kernel; keep drain + one barrier.
    def _cheap_drain_and_barrier(self, tick_clock, wait_clock):
        drain_inst = self.nc.sync.drain()
        wait_clock.add_sem_waits(
            drain_inst.ins, tile.ScopedClock({None: tick_clock.global_clock})
        )
        popped = self.nc._tile_sem_poison_stack.pop()
        assert popped is self._sem_poison

    f32 = mybir.dt.float32
    Alu = mybir.AluOpType
    Act = mybir.ActivationFunctionType

    # fused custom DVE op: out = wrap(in0 + in1 + s0) into [-s1, s1] with
    # period 1 (turn space) -- replaces gpsimd add + add_range_wrap pair
    import concourse.dve_ops as dve_ops

    if not hasattr(dve_ops, "ADD_WRAP_EWALD"):
        from concourse.dve_spec import C0, C1, Spec, Src0, Src1, lower
        from concourse.dve_uop import DveOpSpec

        _y = (Src0 + Src1) + C0

        def _ref(in0, in1, s0, s1, imm2):
            y = in0 + in1 + s0
            return y + (
                (y < -s1).astype(np.float32) - (y > s1).astype(np.float32)
            )

        _spec = Spec(body=_y + ((_y < -C1) - (_y > C1)), reference=_ref)
        _shas = {
            ver: DveOpSpec(
                name="ADD_WRAP_EWALD", opcode=0,
                uops=lower(_spec, ver=ver), rd1_en=True,
            ).sha(ver)
            for ver in ("v3", "v4")
        }
        _op = dve_ops.DveOp("ADD_WRAP_EWALD", _spec, subdim=False, uops_sha=_shas)
        dve_ops.OPS.append(_op)
        dve_ops._SUB_OPCODE_FOR_NAME[_op.name] = (
            dve_ops._CUSTOM_DVE_ROW_BASE + len(dve_ops.OPS) - 1
        )
        dve_ops.CUSTOM_DVE_SPECS[_op.name] = _spec
        dve_ops.ADD_WRAP_EWALD = _op
    AW = dve_ops.ADD_WRAP_EWALD

    f32r = mybir.dt.float32r
    tile.TileContext._drain_and_barrier = _cheap_drain_and_barrier
    nc = bacc.Bacc(None, target_bir_lowering=False)

    # one input tensor: cols 0:24 = phi (chunk-major, 3/chunk), 24:32 = q
    inp = nc.dram_tensor("inp", [128, 3 * CHUNKS + CHUNKS], f32, kind="ExternalInput")
    sout = nc.dram_tensor("sout", [2 * NH, 2 * NPP], f32, kind="ExternalOutput")

    # j values (d-major blocks of 21, col = d*21 + j+10) + col 63 = -2pi
    jdat = np.concatenate(
        [
            np.tile(np.arange(-NK, NK + 1, dtype=np.float32), (128, 3)),
            np.full((128, 1), -TWOPI, np.float32),
        ],
        axis=1,
    )
    jrow = nc.inline_tensor(jdat, name="jrow")

    NW = 3 * NJ                      # 63 cols per chunk in F/th tiles
    NV = 4 * NPP                     # 1768 cols in fused ACT input per pair
    NT = 2 * CHUNKS * NH             # 176 cols of d1 tables (k-major: c1|s1)

    with tile.TileContext(nc) as tc:
        with (
            tc.tile_pool(name="const", bufs=1) as cp,
            tc.tile_pool(name="work", bufs=3) as wp,
            tc.tile_pool(name="psum", bufs=1, space="PSUM") as pp,
        ):
            it = cp.tile([128, 4 * CHUNKS], f32)
            nc.gpsimd.dma_start(out=it[:], in_=inp[:])
            jt = cp.tile([128, NW + 1], f32)
            nc.sync.dma_start(out=jt[:], in_=jrow[:])
            cm2pi = jt[:, NW : NW + 1]

            ps_r = pp.tile([2 * NH, NPP], f32)
            ps_i = pp.tile([2 * NH, NPP], f32)

            # stage 0 in two halves: theta' = j*phi'; F = round - theta'
            HC = CHUNKS // 2
            tha = cp.tile([128, CHUNKS * NW], f32)
            t1a = cp.tile([128, CHUNKS * NW], f32)
            Fa = cp.tile([128, CHUNKS * NW], f32)
            for h in range(2):
                hs, he = h * HC * NW, (h + 1) * HC * NW
                nc.vector.tensor_tensor(
                    out=tha[:, hs:he].rearrange("p (t d j) -> p t d j", t=HC, d=3),
                    in0=it[:, 3 * h * HC : 3 * (h + 1) * HC]
                    .rearrange("p (t d) -> p t d", d=3)
                    .unsqueeze(3)
                    .broadcast_to([128, HC, 3, NJ]),
                    in1=jt[:, 0:NW]
                    .rearrange("p (d j) -> p d j", d=3)
                    .unsqueeze(1)
                    .broadcast_to([128, HC, 3, NJ]),
                    op=Alu.mult,
                )
                nc.vector.tensor_scalar(
                    out=t1a[:, hs:he], in0=tha[:, hs:he], scalar1=MAGIC,
                    scalar2=None, op0=Alu.add,
                )
                nc.vector.scalar_tensor_tensor(
                    out=Fa[:, hs:he], in0=t1a[:, hs:he], scalar=-MAGIC,
                    in1=tha[:, hs:he], op0=Alu.add, op1=Alu.subtract,
                )

            Fv = Fa[:].rearrange("p (t w) -> p t w", t=CHUNKS)  # [128, 8, 63]

            # d1 tables, (t, k, j) interleaved: cols 22t+j = c1, 22t+11+j = s1
            F1a = Fv[:, :, NK : NK + NH]                       # [128, 8, 11]
            VT = cp.tile([128, NT], f32)
            TT = cp.tile([128, NT], f32)
            lhsTa = cp.tile([128, NT], f32r)
            tkj = lambda ap: ap.rearrange("p (t k j) -> p t k j", t=CHUNKS, k=2)
            nc.vector.add_range_wrap(
                out=tkj(VT[:])[:, :, 0, :], in_=F1a, shift=-0.25,
                bound=0.5, period=1.0,
            )
            nc.scalar.activation(
                out=tkj(TT[:])[:, :, 1, :], in_=F1a, func=Act.Sin,
                bias=0.0, scale=cm2pi,
            )
            nc.scalar.activation(
                out=tkj(TT[:])[:, :, 0, :], in_=tkj(VT[:])[:, :, 0, :],
                func=Act.Sin, bias=0.0, scale=cm2pi,
            )
            nc.gpsimd.tensor_tensor(
                out=tkj(lhsTa[:]),
                in0=tkj(TT[:]),
                in1=it[:, 3 * CHUNKS : 4 * CHUNKS]
                .unsqueeze(2)
                .unsqueeze(3)
                .broadcast_to([128, CHUNKS, 2, NH]),
                op=Alu.mult,
            )

            for t in range(CHUNKS):
                F2bc = Fv[:, t, NJ : 2 * NJ].unsqueeze(2).broadcast_to(
                    [128, NJ, NJ]
                )
                F3bc = Fv[:, t, 2 * NJ : 3 * NJ].unsqueeze(1).broadcast_to(
                    [128, NJ, NJ]
                )
                # V = [A_i src (442) | A_r src (442)]; col 441 of each
                # block is pad (fp32r needs even counts); host ignores it.
                # Each block = wrap(F2 (+) F3 + shift) fused in one DVE op.
                V = wp.tile([128, 2 * NPP], f32)
                Vb = V[:].rearrange("p (blk w) -> p blk w", blk=2)
                nc.gpsimd.memset(Vb[:, :, NPAIR:NPP], 0.0)
                nc.vector._custom_dve(
                    AW, out=Vb[:, 0, 0:NPAIR].rearrange("p (a b) -> p a b", a=NJ),
                    in0=F2bc, in1=F3bc, s0=0.0, s1=0.5,
                )
                nc.vector._custom_dve(
                    AW, out=Vb[:, 1, 0:NPAIR].rearrange("p (a b) -> p a b", a=NJ),
                    in0=F2bc, in1=F3bc, s0=-0.25, s1=0.5,
                )
                # Sin(-2pi*v) -> [A_i | A_r]
                AA = wp.tile([128, 2 * NPP], f32r)
                nc.scalar.activation(
                    out=AA[:], in_=V[:], func=Act.Sin, bias=0.0, scale=cm2pi
                )
                lh = lhsTa[:, 2 * NH * t : 2 * NH * (t + 1)]
                nc.tensor.matmul(
                    out=ps_i[:], lhsT=lh, rhs=AA[:, 0:NPP],
                    start=(t == 0), stop=(t == CHUNKS - 1),
                )
                nc.tensor.matmul(
                    out=ps_r[:], lhsT=lh, rhs=AA[:, NPP : 2 * NPP],
                    start=(t == 0), stop=(t == CHUNKS - 1),
                )

            # PSUM -> SBUF -> DRAM (combine happens on host)
            so = wp.tile([2 * NH, 2 * NPP], f32)
            nc.vector.tensor_copy(out=so[:, 0:NPP], in_=ps_r[:])
            nc.scalar.activation(
                out=so[:, NPP : 2 * NPP], in_=ps_i[:], func=Act.Copy
            )
            nc.sync.dma_start(out=sout[:, 0:NPP], in_=so[:, 0:NPP])
            nc.sync.dma_start(out=sout[:, NPP : 2 * NPP], in_=so[:, NPP : 2 * NPP])

    nc.compile()
    return nc


def _get_nc():
    if "nc" not in _CACHE:
        _CACHE["nc"] = _build_nc()
    return _CACHE["nc"]


def _host_inputs(q, r, cell):
    """Per-core phi (reduced turns) and q in SBUF layout."""
    in_maps = []
    for c in range(N_CORES):
        b = c // CORES_PER_SYS
        half = c % CORES_PER_SYS
        lo = b * N_PER + half * ATOMS_PER_CORE
        rs = r[lo : lo + ATOMS_PER_CORE].astype(np.float64)
        qs = q[lo : lo + ATOMS_PER_CORE, 0].astype(np.float32)
        minv = np.linalg.inv(cell[b].astype(np.float64))
        phi = (rs @ minv) % 1.0                      # [1000, 3] turns in [0,1)
        phi_p = np.zeros((PADN, 3), np.float32)
        phi_p[:ATOMS_PER_CORE] = phi.astype(np.float32)
        q_p = np.zeros((PADN,), np.float32)
        q_p[:ATOMS_PER_CORE] = qs
        # atom (t*128+p) -> [p, t*3+d] and [p, 24+t]
        inp = np.zeros((128, 4 * CHUNKS), np.float32)
        inp[:, 0 : 3 * CHUNKS] = (
            phi_p.reshape(CHUNKS, 128, 3).transpose(1, 0, 2).reshape(128, CHUNKS * 3)
        )
        inp[:, 3 * CHUNKS :] = q_p.reshape(CHUNKS, 128).T
        in_maps.append({"inp": inp})
    return in_maps


def _host_weights(cell):
    """w[b, n1(0..10), n2, n3] = mask * 2 * kfac / V, mirroring reference."""
    k_sq_max = (TWOPI / DL) ** 2
    sigma_sq_half = SIGMA ** 2 / 2.0
    rng = np.arange(-NK, NK + 1, dtype=np.float64)
    n1, n2, n3 = np.meshgrid(rng[NK:], rng, rng, indexing="ij")  # n1 >= 0
    nvec = np.stack([n1.ravel(), n2.ravel(), n3.ravel()], axis=1)  # [NH*441, 3]
    hemi = (
        (nvec[:, 0] > 0)
        | ((nvec[:, 0] == 0) & (nvec[:, 1] > 0))
        | ((nvec[:, 0] == 0) & (nvec[:, 1] == 0) & (nvec[:, 2] > 0))
    )
    ws = []
    for b in range(B):
        cb = cell[b].astype(np.float64)
        G = TWOPI * np.linalg.inv(cb).T
        kvec = nvec @ G
        k_sq = np.sum(kvec ** 2, axis=1)
        mask = (k_sq > 0) & (k_sq <= k_sq_max) & hemi
        kfac = np.exp(-sigma_sq_half * k_sq) / (k_sq + EPS)
        vol = np.linalg.det(cb)
        ws.append(np.where(mask, 2.0 * kfac, 0.0) / vol)
    return np.stack(ws).reshape(B, NH, NPAIR)


def kernel(q, r, cell, batch):
    from concourse.bass_utils import run_bass_kernel_spmd

    q = np.asarray(q)
    r = np.asarray(r)
    cell = np.asarray(cell)

    nc = _get_nc()
    in_maps = _host_inputs(q, r, cell)
    res = run_bass_kernel_spmd(nc, in_maps, core_ids=list(range(N_CORES))).results

    w = _host_weights(cell)
    pot = np.zeros(B, np.float64)
    for b in range(B):
        s_r = np.zeros((NH, NPAIR), np.float64)
        s_i = np.zeros_like(s_r)
        for half in range(CORES_PER_SYS):
            o = res[b * CORES_PER_SYS + half]["sout"].astype(np.float64)
            P, Q = o[0:NH, 0:NPAIR], o[NH : 2 * NH, 0:NPAIR]
            R, T = o[0:NH, NPP : NPP + NPAIR], o[NH : 2 * NH, NPP : NPP + NPAIR]
            s_r += P - T
            s_i += R + Q
        s_sq = s_r ** 2 + s_i ** 2
        qb = q[b * N_PER : (b + 1) * N_PER, 0].astype(np.float64)
        self_e = np.sum(qb ** 2) / (SIGMA * TWOPI ** 1.5)
        pot[b] = (np.sum(w[b] * s_sq) - self_e) * NORM
    return pot.astype(np.float32)



# revision 4
# speedup vs baseline: 1.4324x; 1.4324x over previous
"""Ewald reciprocal-space sum on 8 Trainium2 NeuronCores.

Math: for each system b, S(k) = sum_n q_n e^{i k.r_n} over the static
integer k-grid n in [-10,10]^3, k = n @ G, G = 2*pi*inv(cell)^T.
Key identity: k.r = n1*phi1 + n2*phi2 + n3*phi3 with phi_d = G_d . r,
so e^{i k.r} factorizes into per-dimension phase tables.

Conjugate symmetry: |S(-k)| = |S(k)|, so it suffices to compute S on
the half pair-grid n2 in [0,10] x n3 in [-10,10] (231 pairs) for the
FULL n1 range [-10,10]; the reference hemisphere maps onto this grid
via (n1,n2,n3) -> (-n1,-n2,-n3) when n2<0 or (n2==0 and n3<0).

Device work per core (SPMD, core c owns half the atoms of system c//2):
  - per-dim tables F = round(j*phi) - j*phi == -j*phi (mod 1) via one
    fused custom DVE op (FRACMUL); shifted variants (for cos) via
    FRACMULS = same with +0.25 added before rounding
  - per chunk, ONE custom DVE add-wrap over [F3 | F3-.25] gives both
    sin- and cos-variant pair angles in one 462-col pass
  - ACT Sin (scale=-2pi) turns angle tiles into bf16 tables
  - lhsT = q * [cos(n1 phi1) | sin(n1 phi1)]  (Pool multiply)
  - S partial = lhsT^T @ pairtable via 8 PSUM-accumulated bf16
    matmuls -> ps[42, 462]
Host: O(B*K) weight mask + final reduction, summing partial S across
the core pair before squaring.
"""

import numpy as np

# ---- problem constants (hardcoded per contract) ----
B = 4
N_PER = 2000
NK = 10                      # k-grid extent: n in [-NK, NK]
NJ = 2 * NK + 1              # 21
NH = NK + 1                  # 11 non-negative n2 values
NPAIR = NH * 2 * NJ          # 462 pair cols per chunk: (j2, [sin|cos], j3)
DL = 2.0
SIGMA = 1.0
EPS = 1e-6
NORM = 90.0474
TWOPI = 2.0 * np.pi

MAGIC = 12582912.0           # 1.5 * 2**23: fp32 round-to-nearest trick

N_CORES = 8
CORES_PER_SYS = 2
ATOMS_PER_CORE = (B * N_PER) // N_CORES     # 1000
CHUNKS = 8                                  # ceil(1000/128)
PADN = CHUNKS * 128                         # 1024
GRP = 2                                     # chunks per ACT/matmul group

LW = 2 * NJ                  # 42 lhs cols per chunk (cos1 | sin1)
LWP = LW + 2                 # 44: padded stride, keeps 8B alignment

_CACHE = {}


def _register_dve_ops():
    import concourse.dve_ops as dve_ops
    from concourse.dve_spec import C0, C1, Spec, Src0, Src1, lower
    from concourse.dve_uop import DveOpSpec

    def _register(name, spec):
        shas = {
            ver: DveOpSpec(
                name=name, opcode=0, uops=lower(spec, ver=ver), rd1_en=True,
            ).sha(ver)
            for ver in ("v3", "v4")
        }
        op = dve_ops.DveOp(name, spec, subdim=False, uops_sha=shas)
        dve_ops.OPS.append(op)
        dve_ops._SUB_OPCODE_FOR_NAME[name] = (
            dve_ops._CUSTOM_DVE_ROW_BASE + len(dve_ops.OPS) - 1
        )
        dve_ops.CUSTOM_DVE_SPECS[name] = spec
        setattr(dve_ops, name, op)
        return op

    if not hasattr(dve_ops, "ADD_WRAP_EWALD"):
        _y = (Src0 + Src1) + C0

        def _ref(in0, in1, s0, s1, imm2):
            y = in0 + in1 + s0
            return y + (
                (y < -s1).astype(np.float32) - (y > s1).astype(np.float32)
            )

        _register("ADD_WRAP_EWALD", Spec(body=_y + ((_y < -C1) - (_y > C1)),
                                         reference=_ref))

    if not hasattr(dve_ops, "FRACMUL_EWALD"):
        _t = (Src0 * Src1) + C1

        def _reff(in0, in1, s0, s1, imm2):
            t = in0 * in1 + s1
            return ((t + s0) - s0) - t

        _register("FRACMUL_EWALD", Spec(body=((_t + C0) - C0) - _t,
                                        reference=_reff))

    return dve_ops.ADD_WRAP_EWALD, dve_ops.FRACMUL_EWALD


def _build_nc():
    import concourse.bacc as bacc
    import concourse.mybir as mybir
    import concourse.tile as tile

    # cheaper TileContext exit: the Bass preamble re-clears the whole
    # kernel sem range at every execution, so the exit-time sem clear and
    # second all-engine barrier are redundant for this single-context
    # kernel; keep drain + one barrier.
    def _cheap_drain_and_barrier(self, tick_clock, wait_clock):
        drain_inst = self.nc.sync.drain()
        wait_clock.add_sem_waits(
            drain_inst.ins, tile.ScopedClock({None: tick_clock.global_clock})
        )
        popped = self.nc._tile_sem_poison_stack.pop()
        assert popped is self._sem_poison

    f32 = mybir.dt.float32
    bf16 = mybir.dt.bfloat16
    Act = mybir.ActivationFunctionType
    Alu = mybir.AluOpType
    AW, FM = _register_dve_ops()

    tile.TileContext._drain_and_barrier = _cheap_drain_and_barrier
    nc = bacc.Bacc(None, target_bir_lowering=False)

    # input layout per core, [128, 54]:
    #   0:8    phi1 per chunk      8:16  phi2      16:24  phi3
    #   24:32  q per chunk
    #   32:53  j values -10..10 (col 32+jj has value jj-10)
    #   53     0.0
    inp = nc.dram_tensor("inp", [128, 54], f32, kind="ExternalInput")
    sout = nc.dram_tensor("sout", [LW, NPAIR], f32, kind="ExternalOutput")

    NGR = CHUNKS // GRP
    with tile.TileContext(nc) as tc:
        with (
            tc.tile_pool(name="c", bufs=1) as cp,
            tc.tile_pool(name="ps", bufs=1, space="PSUM") as pp,
        ):
            it = cp.tile([128, 54], f32)
            scr = cp.tile([128, 2], f32)
            F12 = cp.tile([128, 2 * CHUNKS * NJ], f32)   # F1 | F2 (t,j)-major
            Vd1 = cp.tile([128, CHUNKS * NJ], f32)       # wrap(F1 - .25)
            F3X = cp.tile([128, CHUNKS * 2 * NJ], f32)   # per chunk [F3|F3-.25]
            T = cp.tile([128, CHUNKS * LWP], f32)
            lhsT = cp.tile([128, CHUNKS * LWP], bf16)
            V = [cp.tile([128, GRP * NPAIR], f32, name=f"V{g}")
                 for g in range(NGR)]
            AA = [cp.tile([128, GRP * NPAIR], bf16, name=f"AA{g}")
                  for g in range(NGR)]
            ps = pp.tile([LW, NPAIR], f32)
            so = cp.tile([LW, NPAIR], f32)

            # input DMA first thing on the idle sync queue
            nc.sync.dma_start(out=it[:], in_=inp[:])

            # dummy Sin on a const AP: hoists the ACT table load so it
            # overlaps the input DMA instead of sitting on the critical path
            zc = nc.const_aps.aps[(f32, 0.0)]
            nc.scalar.activation(out=scr[:, 0:1], in_=zc, func=Act.Sin,
                                 bias=0.0, scale=1.0)

            j_bc8 = it[:, 32:53].unsqueeze(1).broadcast_to([128, CHUNKS, NJ])
            j_bc16 = it[:, 32:53].unsqueeze(1).broadcast_to([128, 2 * CHUNKS, NJ])

            # F12 = -j*phi_{1,2} (mod 1);  Vd1 = F1 - .25 (mod 1)
            nc.vector._custom_dve(
                FM, out=F12[:].rearrange("p (m j) -> p m j", j=NJ),
                in0=it[:, 0:16].unsqueeze(2).broadcast_to([128, 2 * CHUNKS, NJ]),
                in1=j_bc16, s0=MAGIC, s1=0.0,
            )
            nc.vector._custom_dve(
                FM, out=Vd1[:].rearrange("p (t j) -> p t j", j=NJ),
                in0=it[:, 0:8].unsqueeze(2).broadcast_to([128, CHUNKS, NJ]),
                in1=j_bc8, s0=MAGIC, s1=0.25,
            )
            F3Xv = F3X[:].rearrange("p (t v j) -> p t v j", t=CHUNKS, v=2)
            for v, sh in ((0, 0.0), (1, 0.25)):
                nc.vector._custom_dve(
                    FM, out=F3Xv[:, :, v, :],
                    in0=it[:, 16:24].unsqueeze(2).broadcast_to([128, CHUNKS, NJ]),
                    in1=j_bc8, s0=MAGIC, s1=sh,
                )

            # d1 tables: cos = Sin(-2pi * Vd1), sin = Sin(-2pi * F1)
            Tv = T[:].rearrange("p (t w) -> p t w", t=CHUNKS)
            nc.scalar.activation(
                out=Tv[:, :, 0:NJ],
                in_=Vd1[:].rearrange("p (t j) -> p t j", j=NJ),
                func=Act.Sin, bias=0.0, scale=-TWOPI,
            )
            nc.scalar.activation(
                out=Tv[:, :, NJ:LW],
                in_=F12[:, 0 : CHUNKS * NJ].rearrange("p (t j) -> p t j", j=NJ),
                func=Act.Sin, bias=0.0, scale=-TWOPI,
            )
            # lhsT = q * T   (Pool engine; q broadcast along the 42 cols)
            q_bc = it[:, 24:32].unsqueeze(2).broadcast_to([128, CHUNKS, LW])
            nc.gpsimd.tensor_tensor(
                out=lhsT[:].rearrange("p (t w) -> p t w", t=CHUNKS)[:, :, 0:LW],
                in0=Tv[:, :, 0:LW], in1=q_bc, op=Alu.mult,
            )

            for g in range(NGR):
                for i in range(GRP):
                    c = g * GRP + i
                    # pair angles: wrap(F2[j2] + F3X[v,j3]) for j2 in 0..10,
                    # v in {sin, cos}, j3 in -10..10  -> [128, 11, 42]
                    f2 = (
                        F12[:, CHUNKS * NJ + c * NJ + NK : CHUNKS * NJ + (c + 1) * NJ]
                        .unsqueeze(2)
                        .broadcast_to([128, NH, 2 * NJ])
                    )
                    f3 = (
                        F3X[:, c * 2 * NJ : (c + 1) * 2 * NJ]
                        .unsqueeze(1)
                        .broadcast_to([128, NH, 2 * NJ])
                    )
                    nc.vector._custom_dve(
                        AW,
                        out=V[g][:, i * NPAIR : (i + 1) * NPAIR].rearrange(
                            "p (a b) -> p a b", a=NH
                        ),
                        in0=f2, in1=f3, s0=0.0, s1=0.5,
                    )
                nc.scalar.activation(out=AA[g][:], in_=V[g][:], func=Act.Sin,
                                     bias=0.0, scale=-TWOPI)
                for i in range(GRP):
                    c = g * GRP + i
                    nc.tensor.matmul(
                        out=ps[:],
                        lhsT=lhsT[:, c * LWP : c * LWP + LW],
                        rhs=AA[g][:, i * NPAIR : (i + 1) * NPAIR],
                        start=(c == 0), stop=(c == CHUNKS - 1),
                    )

            nc.scalar.activation(out=so[:], in_=ps[:], func=Act.Copy)
            nc.sync.dma_start(out=sout[:], in_=so[:])

    nc.compile()
    return nc


def _get_nc():
    if "nc" not in _CACHE:
        _CACHE["nc"] = _build_nc()
    return _CACHE["nc"]


def _host_inputs(q, r, cell):
    """Per-core phi (reduced turns), q, and j constants in SBUF layout."""
    in_maps = []
    for c in range(N_CORES):
        b = c // CORES_PER_SYS
        half = c % CORES_PER_SYS
        lo = b * N_PER + half * ATOMS_PER_CORE
        rs = r[lo : lo + ATOMS_PER_CORE].astype(np.float64)
        qs = q[lo : lo + ATOMS_PER_CORE, 0].astype(np.float32)
        minv = np.linalg.inv(cell[b].astype(np.float64))
        phi = (rs @ minv) % 1.0                      # [1000, 3] turns in [0,1)
        phi_p = np.zeros((PADN, 3), np.float32)
        phi_p[:ATOMS_PER_CORE] = phi.astype(np.float32)
        q_p = np.zeros((PADN,), np.float32)
        q_p[:ATOMS_PER_CORE] = qs
        inp = np.zeros((128, 54), np.float32)
        # phi d-major: col d*8 + t for atom (t*128 + p)
        inp[:, 0:24] = (
            phi_p.reshape(CHUNKS, 128, 3).transpose(1, 2, 0).reshape(128, 24)
        )
        inp[:, 24:32] = q_p.reshape(CHUNKS, 128).T
        inp[:, 32:53] = np.arange(-NK, NK + 1, dtype=np.float32)[None, :]
        in_maps.append({"inp": inp})
    return in_maps


def _host_weights(cell):
    """w[b, n1(-10..10), n2(0..10), n3(-10..10)]: reference hemisphere
    weights 2*kfac/V folded onto the half pair-grid via k -> -k."""
    k_sq_max = (TWOPI / DL) ** 2
    sigma_sq_half = SIGMA ** 2 / 2.0
    rng = np.arange(-NK, NK + 1, dtype=np.float64)
    n1, n2, n3 = np.meshgrid(rng, rng, rng, indexing="ij")
    nvec = np.stack([n1.ravel(), n2.ravel(), n3.ravel()], axis=1)
    hemi = (
        (nvec[:, 0] > 0)
        | ((nvec[:, 0] == 0) & (nvec[:, 1] > 0))
        | ((nvec[:, 0] == 0) & (nvec[:, 1] == 0) & (nvec[:, 2] > 0))
    )
    ws = []
    for b in range(B):
        cb = cell[b].astype(np.float64)
        G = TWOPI * np.linalg.inv(cb).T
        kvec = nvec @ G
        k_sq = np.sum(kvec ** 2, axis=1)
        mask = (k_sq > 0) & (k_sq <= k_sq_max) & hemi
        kfac = np.exp(-sigma_sq_half * k_sq) / (k_sq + EPS)
        vol = np.linalg.det(cb)
        wk = np.where(mask, 2.0 * kfac, 0.0) / vol
        wg = np.zeros((NJ, NH, NJ), np.float64)
        idx = 0
        for i1 in range(-NK, NK + 1):
            for i2 in range(-NK, NK + 1):
                for i3 in range(-NK, NK + 1):
                    w = wk[idx]
                    idx += 1
                    if w == 0.0:
                        continue
                    if (i2 > 0) or (i2 == 0 and i3 >= 0):
                        wg[i1 + NK, i2, i3 + NK] += w
                    else:
                        wg[-i1 + NK, -i2, -i3 + NK] += w
        ws.append(wg)
    return np.stack(ws)


def kernel(q, r, cell, batch):
    from concourse.bass_utils import run_bass_kernel_spmd

    q = np.asarray(q)
    r = np.asarray(r)
    cell = np.asarray(cell)

    nc = _get_nc()
    in_maps = _host_inputs(q, r, cell)
    res = run_bass_kernel_spmd(nc, in_maps, core_ids=list(range(N_CORES))).results

    w = _host_weights(cell)
    pot = np.zeros(B, np.float64)
    for b in range(B):
        s_r = np.zeros((NJ, NH, NJ), np.float64)
        s_i = np.zeros_like(s_r)
        for half in range(CORES_PER_SYS):
            o = res[b * CORES_PER_SYS + half]["sout"].astype(np.float64)
            # rows 0:21 = cos1 (n1=-10..10), 21:42 = sin1
            # cols: (j2, [sinP | cosP], j3) -> [42, 11, 2, 21]
            o4 = o.reshape(LW, NH, 2, NJ)
            M_cs = o4[0:NJ, :, 0, :]          # cos1 . sinP
            M_ss = o4[NJ:LW, :, 0, :]         # sin1 . sinP
            M_cc = o4[0:NJ, :, 1, :]          # cos1 . cosP
            M_sc = o4[NJ:LW, :, 1, :]         # sin1 . cosP
            s_r += M_cc - M_ss
            s_i += M_cs + M_sc
        s_sq = s_r ** 2 + s_i ** 2
        qb = q[b * N_PER : (b + 1) * N_PER, 0].astype(np.float64)
        self_e = np.sum(qb ** 2) / (SIGMA * TWOPI ** 1.5)
        pot[b] = (np.sum(w[b] * s_sq) - self_e) * NORM
    return pot.astype(np.float32)


# revision 8
# speedup vs baseline: 1.4363x; 1.0027x over previous
"""Ewald reciprocal-space sum on 8 Trainium2 NeuronCores.

Math: for each system b, S(k) = sum_n q_n e^{i k.r_n} over the static
integer k-grid n in [-10,10]^3, k = n @ G, G = 2*pi*inv(cell)^T.
Key identity: k.r = n1*phi1 + n2*phi2 + n3*phi3 with phi_d = G_d . r,
so e^{i k.r} factorizes into per-dimension phase tables.

Conjugate symmetry: |S(-k)| = |S(k)|, so it suffices to compute S on
the half pair-grid n2 in [0,10] x n3 in [-10,10] (231 pairs) for the
FULL n1 range [-10,10]; the reference hemisphere maps onto this grid
via (n1,n2,n3) -> (-n1,-n2,-n3) when n2<0 or (n2==0 and n3<0).

Device work per core (SPMD, core c owns half the atoms of system c//2):
  - per-dim tables F = round(j*phi) - j*phi == -j*phi (mod 1) via one
    fused custom DVE op (FRACMUL); shifted variants (for cos) via
    FRACMULS = same with +0.25 added before rounding
  - per chunk, ONE custom DVE add-wrap over [F3 | F3-.25] gives both
    sin- and cos-variant pair angles in one 462-col pass
  - ACT Sin (scale=-2pi) turns angle tiles into bf16 tables
  - lhsT = q * [cos(n1 phi1) | sin(n1 phi1)]  (Pool multiply)
  - S partial = lhsT^T @ pairtable via 8 PSUM-accumulated bf16
    matmuls -> ps[42, 462]
Host: O(B*K) weight mask + final reduction, summing partial S across
the core pair before squaring.
"""

import numpy as np

# ---- problem constants (hardcoded per contract) ----
B = 4
N_PER = 2000
NK = 10                      # k-grid extent: n in [-NK, NK]
NJ = 2 * NK + 1              # 21
NH = NK + 1                  # 11 non-negative n2 values
NPAIR = NH * 2 * NJ          # 462 pair cols per chunk: (j2, [sin|cos], j3)
DL = 2.0
SIGMA = 1.0
EPS = 1e-6
NORM = 90.0474
TWOPI = 2.0 * np.pi

MAGIC = 12582912.0           # 1.5 * 2**23: fp32 round-to-nearest trick

N_CORES = 8
CORES_PER_SYS = 2
ATOMS_PER_CORE = (B * N_PER) // N_CORES     # 1000
CHUNKS = 8                                  # ceil(1000/128)
PADN = CHUNKS * 128                         # 1024
GRP = 2                                     # chunks per ACT/matmul group

LW = 2 * NJ                  # 42 lhs cols per chunk (cos1 | sin1)
LWP = LW + 2                 # 44: padded stride, keeps 8B alignment

_CACHE = {}


def _register_dve_ops():
    import concourse.dve_ops as dve_ops
    from concourse.dve_spec import C0, C1, Spec, Src0, Src1, lower
    from concourse.dve_uop import DveOpSpec

    def _register(name, spec):
        shas = {
            ver: DveOpSpec(
                name=name, opcode=0, uops=lower(spec, ver=ver), rd1_en=True,
            ).sha(ver)
            for ver in ("v3", "v4")
        }
        op = dve_ops.DveOp(name, spec, subdim=False, uops_sha=shas)
        dve_ops.OPS.append(op)
        dve_ops._SUB_OPCODE_FOR_NAME[name] = (
            dve_ops._CUSTOM_DVE_ROW_BASE + len(dve_ops.OPS) - 1
        )
        dve_ops.CUSTOM_DVE_SPECS[name] = spec
        setattr(dve_ops, name, op)
        return op

    if not hasattr(dve_ops, "ADD_WRAP_EWALD"):
        _y = (Src0 + Src1) + C0

        def _ref(in0, in1, s0, s1, imm2):
            y = in0 + in1 + s0
            return y + (
                (y < -s1).astype(np.float32) - (y > s1).astype(np.float32)
            )

        _register("ADD_WRAP_EWALD", Spec(body=_y + ((_y < -C1) - (_y > C1)),
                                         reference=_ref))

    if not hasattr(dve_ops, "FRACMUL_EWALD"):
        _t = (Src0 * Src1) + C1

        def _reff(in0, in1, s0, s1, imm2):
            t = in0 * in1 + s1
            return ((t + s0) - s0) - t

        _register("FRACMUL_EWALD", Spec(body=((_t + C0) - C0) - _t,
                                        reference=_reff))

    return dve_ops.ADD_WRAP_EWALD, dve_ops.FRACMUL_EWALD


def _build_nc():
    import concourse.bacc as bacc
    import concourse.mybir as mybir
    import concourse.tile as tile

    # cheaper TileContext exit: the Bass preamble re-clears the whole
    # kernel sem range at every execution, so the exit-time sem clear and
    # second all-engine barrier are redundant for this single-context
    # kernel; keep drain + one barrier.
    def _cheap_drain_and_barrier(self, tick_clock, wait_clock):
        drain_inst = self.nc.sync.drain()
        wait_clock.add_sem_waits(
            drain_inst.ins, tile.ScopedClock({None: tick_clock.global_clock})
        )
        popped = self.nc._tile_sem_poison_stack.pop()
        assert popped is self._sem_poison

    f32 = mybir.dt.float32
    bf16 = mybir.dt.bfloat16
    Act = mybir.ActivationFunctionType
    Alu = mybir.AluOpType
    AW, FM = _register_dve_ops()

    # The Bass preamble clears the whole kernel sem range one sem at a
    # time at the end of every execution (~23ns/sem/engine + dispatch);
    # shrink the range so the tail clear covers 48 sems instead of 253.
    import concourse.bass as bass_mod

    if not hasattr(bass_mod, "_orig_sem_range"):
        bass_mod._orig_sem_range = bass_mod.get_kernel_semaphore_range
        bass_mod.get_kernel_semaphore_range = lambda: range(
            bass_mod._orig_sem_range().start, bass_mod._orig_sem_range().start + 48
        )

    tile.TileContext._drain_and_barrier = _cheap_drain_and_barrier
    nc = bacc.Bacc(None, target_bir_lowering=False)

    # input layout per core, [128, 54]:
    #   0:8    phi1 per chunk      8:16  phi2      16:24  phi3
    #   24:32  q per chunk
    #   32:53  j values -10..10 (col 32+jj has value jj-10)
    #   53     0.0
    inp = nc.dram_tensor("inp", [128, 54], f32, kind="ExternalInput")
    sout = nc.dram_tensor("sout", [LW, NPAIR], f32, kind="ExternalOutput")

    NGR = CHUNKS // GRP
    with tile.TileContext(nc) as tc:
        with (
            tc.tile_pool(name="c", bufs=1) as cp,
            tc.tile_pool(name="ps", bufs=1, space="PSUM") as pp,
        ):
            it = cp.tile([128, 54], f32)
            scr = cp.tile([128, 2], f32)
            F12 = cp.tile([128, 2 * CHUNKS * NJ], f32)   # F1 | F2 (t,j)-major
            Vd1 = cp.tile([128, CHUNKS * NJ], f32)       # wrap(F1 - .25)
            F3X = cp.tile([128, CHUNKS * 2 * NJ], f32)   # per chunk [F3|F3-.25]
            T = cp.tile([128, CHUNKS * LWP], f32)
            lhsT = cp.tile([128, CHUNKS * LWP], bf16)
            V = [cp.tile([128, GRP * NPAIR], f32, name=f"V{g}")
                 for g in range(NGR)]
            AA = [cp.tile([128, GRP * NPAIR], bf16, name=f"AA{g}")
                  for g in range(NGR)]
            ps = pp.tile([LW, NPAIR], f32)
            so = cp.tile([LW, NPAIR], f32)

            # input DMA first thing on the idle sync queue
            nc.sync.dma_start(out=it[:], in_=inp[:])

            # dummy Sin on a const AP: hoists the ACT table load so it
            # overlaps the input DMA instead of sitting on the critical path
            zc = nc.const_aps.aps[(f32, 0.0)]
            nc.scalar.activation(out=scr[:, 0:1], in_=zc, func=Act.Sin,
                                 bias=0.0, scale=1.0)

            j_bc8 = it[:, 32:53].unsqueeze(1).broadcast_to([128, CHUNKS, NJ])
            j_bc16 = it[:, 32:53].unsqueeze(1).broadcast_to([128, 2 * CHUNKS, NJ])

            # F12 = -j*phi_{1,2} (mod 1);  Vd1 = F1 - .25 (mod 1)
            nc.vector._custom_dve(
                FM, out=F12[:].rearrange("p (m j) -> p m j", j=NJ),
                in0=it[:, 0:16].unsqueeze(2).broadcast_to([128, 2 * CHUNKS, NJ]),
                in1=j_bc16, s0=MAGIC, s1=0.0,
            )
            F3Xv = F3X[:].rearrange("p (t v j) -> p t v j", t=CHUNKS, v=2)
            for v, sh in ((0, 0.0), (1, 0.25)):
                nc.vector._custom_dve(
                    FM, out=F3Xv[:, :, v, :],
                    in0=it[:, 16:24].unsqueeze(2).broadcast_to([128, CHUNKS, NJ]),
                    in1=j_bc8, s0=MAGIC, s1=sh,
                )
            nc.vector._custom_dve(
                FM, out=Vd1[:].rearrange("p (t j) -> p t j", j=NJ),
                in0=it[:, 0:8].unsqueeze(2).broadcast_to([128, CHUNKS, NJ]),
                in1=j_bc8, s0=MAGIC, s1=0.25,
            )

            # d1 tables: cos = Sin(-2pi * Vd1), sin = Sin(-2pi * F1)
            Tv = T[:].rearrange("p (t w) -> p t w", t=CHUNKS)
            nc.scalar.activation(
                out=Tv[:, :, 0:NJ],
                in_=Vd1[:].rearrange("p (t j) -> p t j", j=NJ),
                func=Act.Sin, bias=0.0, scale=-TWOPI,
            )
            nc.scalar.activation(
                out=Tv[:, :, NJ:LW],
                in_=F12[:, 0 : CHUNKS * NJ].rearrange("p (t j) -> p t j", j=NJ),
                func=Act.Sin, bias=0.0, scale=-TWOPI,
            )
            # lhsT = q * T   (Pool engine; q broadcast along the 42 cols)
            q_bc = it[:, 24:32].unsqueeze(2).broadcast_to([128, CHUNKS, LW])
            nc.gpsimd.tensor_tensor(
                out=lhsT[:].rearrange("p (t w) -> p t w", t=CHUNKS)[:, :, 0:LW],
                in0=Tv[:, :, 0:LW], in1=q_bc, op=Alu.mult,
            )

            for g in range(NGR):
                for i in range(GRP):
                    c = g * GRP + i
                    # pair angles: wrap(F2[j2] + F3X[v,j3]) for j2 in 0..10,
                    # v in {sin, cos}, j3 in -10..10  -> [128, 11, 42]
                    f2 = (
                        F12[:, CHUNKS * NJ + c * NJ + NK : CHUNKS * NJ + (c + 1) * NJ]
                        .unsqueeze(2)
                        .broadcast_to([128, NH, 2 * NJ])
                    )
                    f3 = (
                        F3X[:, c * 2 * NJ : (c + 1) * 2 * NJ]
                        .unsqueeze(1)
                        .broadcast_to([128, NH, 2 * NJ])
                    )
                    nc.vector._custom_dve(
                        AW,
                        out=V[g][:, i * NPAIR : (i + 1) * NPAIR].rearrange(
                            "p (a b) -> p a b", a=NH
                        ),
                        in0=f2, in1=f3, s0=0.0, s1=0.5,
                    )
                # last group: per-chunk Sin calls so the final matmul can
                # start as soon as the final AW lands (shorter tail)
                nact = GRP if g == NGR - 1 else 1
                for a in range(nact):
                    sl = slice(a * GRP * NPAIR // nact, (a + 1) * GRP * NPAIR // nact)
                    nc.scalar.activation(out=AA[g][:, sl], in_=V[g][:, sl],
                                         func=Act.Sin, bias=0.0, scale=-TWOPI)
                for i in range(GRP):
                    c = g * GRP + i
                    nc.tensor.matmul(
                        out=ps[:],
                        lhsT=lhsT[:, c * LWP : c * LWP + LW],
                        rhs=AA[g][:, i * NPAIR : (i + 1) * NPAIR],
                        start=(c == 0), stop=(c == CHUNKS - 1),
                    )

            nc.scalar.activation(out=so[:], in_=ps[:], func=Act.Copy)
            nc.sync.dma_start(out=sout[:], in_=so[:])

    nc.compile()
    return nc


def _get_nc():
    if "nc" not in _CACHE:
        _CACHE["nc"] = _build_nc()
    return _CACHE["nc"]


def _host_inputs(q, r, cell):
    """Per-core phi (reduced turns), q, and j constants in SBUF layout."""
    in_maps = []
    for c in range(N_CORES):
        b = c // CORES_PER_SYS
        half = c % CORES_PER_SYS
        lo = b * N_PER + half * ATOMS_PER_CORE
        rs = r[lo : lo + ATOMS_PER_CORE].astype(np.float64)
        qs = q[lo : lo + ATOMS_PER_CORE, 0].astype(np.float32)
        minv = np.linalg.inv(cell[b].astype(np.float64))
        phi = (rs @ minv) % 1.0                      # [1000, 3] turns in [0,1)
        phi_p = np.zeros((PADN, 3), np.float32)
        phi_p[:ATOMS_PER_CORE] = phi.astype(np.float32)
        q_p = np.zeros((PADN,), np.float32)
        q_p[:ATOMS_PER_CORE] = qs
        inp = np.zeros((128, 54), np.float32)
        # phi d-major: col d*8 + t for atom (t*128 + p)
        inp[:, 0:24] = (
            phi_p.reshape(CHUNKS, 128, 3).transpose(1, 2, 0).reshape(128, 24)
        )
        inp[:, 24:32] = q_p.reshape(CHUNKS, 128).T
        inp[:, 32:53] = np.arange(-NK, NK + 1, dtype=np.float32)[None, :]
        in_maps.append({"inp": inp})
    return in_maps


def _host_weights(cell):
    """w[b, n1(-10..10), n2(0..10), n3(-10..10)]: reference hemisphere
    weights 2*kfac/V folded onto the half pair-grid via k -> -k."""
    k_sq_max = (TWOPI / DL) ** 2
    sigma_sq_half = SIGMA ** 2 / 2.0
    rng = np.arange(-NK, NK + 1, dtype=np.float64)
    n1, n2, n3 = np.meshgrid(rng, rng, rng, indexing="ij")
    nvec = np.stack([n1.ravel(), n2.ravel(), n3.ravel()], axis=1)
    hemi = (
        (nvec[:, 0] > 0)
        | ((nvec[:, 0] == 0) & (nvec[:, 1] > 0))
        | ((nvec[:, 0] == 0) & (nvec[:, 1] == 0) & (nvec[:, 2] > 0))
    )
    ws = []
    for b in range(B):
        cb = cell[b].astype(np.float64)
        G = TWOPI * np.linalg.inv(cb).T
        kvec = nvec @ G
        k_sq = np.sum(kvec ** 2, axis=1)
        mask = (k_sq > 0) & (k_sq <= k_sq_max) & hemi
        kfac = np.exp(-sigma_sq_half * k_sq) / (k_sq + EPS)
        vol = np.linalg.det(cb)
        wk = np.where(mask, 2.0 * kfac, 0.0) / vol
        wg = np.zeros((NJ, NH, NJ), np.float64)
        idx = 0
        for i1 in range(-NK, NK + 1):
            for i2 in range(-NK, NK + 1):
                for i3 in range(-NK, NK + 1):
                    w = wk[idx]
                    idx += 1
                    if w == 0.0:
                        continue
                    if (i2 > 0) or (i2 == 0 and i3 >= 0):
                        wg[i1 + NK, i2, i3 + NK] += w
                    else:
                        wg[-i1 + NK, -i2, -i3 + NK] += w
        ws.append(wg)
    return np.stack(ws)


def kernel(q, r, cell, batch):
    from concourse.bass_utils import run_bass_kernel_spmd

    q = np.asarray(q)
    r = np.asarray(r)
    cell = np.asarray(cell)

    nc = _get_nc()
    in_maps = _host_inputs(q, r, cell)
    res = run_bass_kernel_spmd(nc, in_maps, core_ids=list(range(N_CORES))).results

    w = _host_weights(cell)
    pot = np.zeros(B, np.float64)
    for b in range(B):
        s_r = np.zeros((NJ, NH, NJ), np.float64)
        s_i = np.zeros_like(s_r)
        for half in range(CORES_PER_SYS):
            o = res[b * CORES_PER_SYS + half]["sout"].astype(np.float64)
            # rows 0:21 = cos1 (n1=-10..10), 21:42 = sin1
            # cols: (j2, [sinP | cosP], j3) -> [42, 11, 2, 21]
            o4 = o.reshape(LW, NH, 2, NJ)
            M_cs = o4[0:NJ, :, 0, :]          # cos1 . sinP
            M_ss = o4[NJ:LW, :, 0, :]         # sin1 . sinP
            M_cc = o4[0:NJ, :, 1, :]          # cos1 . cosP
            M_sc = o4[NJ:LW, :, 1, :]         # sin1 . cosP
            s_r += M_cc - M_ss
            s_i += M_cs + M_sc
        s_sq = s_r ** 2 + s_i ** 2
        qb = q[b * N_PER : (b + 1) * N_PER, 0].astype(np.float64)
        self_e = np.sum(qb ** 2) / (SIGMA * TWOPI ** 1.5)
        pot[b] = (np.sum(w[b] * s_sq) - self_e) * NORM
    return pot.astype(np.float32)
